# revision 22
# baseline (speedup 1.0000x reference)
"""Trainium2 Bass kernel for IntraFrameNet (self-attention + conv head).

Math (per sample b):
  f = curr_features[b].reshape(C, M)                      # C=128, M=4096
  S = f^T f * C^-0.5   (symmetric, [M, M])
  P = softmax(S, axis=-1)
  feats1 = f @ P^T     ([C, M]);  x = [feats1; f]         # [2C, M]
  y = W1 @ x + b1 -> BN(inference) -> leaky_relu(0.01)
  pred = w2 @ y + b2                                      # [1, M]

Device strategy (data-parallel, 1 sample / core, 8 cores):
  The Activation engine is the bottleneck: exp costs 0.833ns/col + 185ns per
  instruction, and an accum_out read adds another 187ns per instruction. So,
  unlike the usual pattern, most exps run WITHOUT accumulation and the
  softmax denominators are row-summed off-ACT:
    - ~107 of the 128 [128,1024] exp tiles: DVE tensor_reduce (1127ns each)
    - the other ~21: ACT accum_out (+187ns each), keeping DVE under ACT.
  (GpSimd/Pool partition reductions return zeros in this runtime, and the
  custom-DVE fused reduce crashes it, so those routes are out.)
  - S chunks [n-chunk, m-super] by PE with f in bf16 (f32r buys nothing:
    matmul cost is out-cols x 0.4167ns regardless).
  - PV: ot[c, m_super] += matmul(lhsT=fT[chunk], rhs=exp tile) over chunks;
    fT built by PE transposes + DVE copies (GpSimd cannot touch PSUM, so the
    idle Pool engine cannot take any of the PSUM-sourced copies).
  - By symmetry of S the row sums equal the column sums, so D[n] indexed by
    n-chunk groups == D[m] for the matching m-super: the D/head pipeline for
    m-group g needs only chunks 8g..8g+7 of the last m-super, emitted
    interleaved into the last super's chunk loop (head for group 3 trails).
  - Head: Dinv broadcast via PE transpose + sel-matmul; fnorm multiplies the
    Dinv broadcast straight out of PSUM; conv1 with BN folded on host;
    leaky relu on DVE mid-loop / fused ACT Prelu for the tail group.
"""

import numpy as np
import ml_dtypes

import concourse.bass as bass
from concourse import bacc
import concourse.mybir as mybir
import concourse.tile as tile
from concourse.bass_utils import run_bass_kernel_spmd
from concourse.masks import make_identity

B, C, H, W = 8, 128, 64, 64
M = H * W          # 4096
NCH = M // 128     # 32 chunks of n
SUP = 1024         # m columns per super-block
NSUP = M // SUP    # 4
CPS = SUP // 128   # 8 chunks per super
SCALE = float(C) ** -0.5
BN_EPS = 1e-5
LEAKY = 0.01

f32 = mybir.dt.float32
f32r = mybir.dt.float32r
bf16 = mybir.dt.bfloat16
AF = mybir.ActivationFunctionType
AX = mybir.AxisListType
OP = mybir.AluOpType


def _act_accum(s, t):
    """Tiles whose D partial rides the exp's accum_out (ACT) instead of a
    DVE reduce (~33 tiles). Chosen so each super's DVE work (reduces + its
    share of copies and interleaved head stages) fits inside that super's
    ACT window; super 3's late chunks all ride ACT so group 3's D is
    complete the moment the exp stream ends (no DVE backlog before the
    tail) and so DVE has room there for the head stages of groups 0-2."""
    if s == 3:
        # the last super runs its chunks in the order [24..31, 0..23]; the
        # late-processed chunks (16..31) ride ACT so both tail groups' D
        # partials complete exactly when the exp stream ends, and half of
        # the rest ride ACT too because DVE also carries the in-loop head
        # stages for groups 0/1 there
        return True
    return t % 6 == 0


def _build():
    nc = bacc.Bacc("TRN2", target_bir_lowering=False)

    f_d = nc.dram_tensor("f", [C, M], bf16, kind="ExternalInput")
    fT_d = nc.dram_tensor("fT", [128, NCH * 128], bf16, kind="ExternalInput")
    w1aT_d = nc.dram_tensor("w1aT", [C, C], f32r, kind="ExternalInput")
    w1bT_d = nc.dram_tensor("w1bT", [C, C], bf16, kind="ExternalInput")
    bhead_d = nc.dram_tensor("bhead", [C, 1], f32, kind="ExternalInput")
    w2T_d = nc.dram_tensor("w2T", [C, 1], f32r, kind="ExternalInput")
    sel_d = nc.dram_tensor("sel", [CPS, CPS * 128], f32r, kind="ExternalInput")
    pred_d = nc.dram_tensor("pred", [1, M], f32, kind="ExternalOutput")

    with tile.TileContext(nc) as tc:
        with (
            tc.tile_pool(name="singles", bufs=1) as singles,
            tc.tile_pool(name="pbufp", bufs=16) as pbufp,
            tc.tile_pool(name="sbm", bufs=3) as sbm,
            tc.tile_pool(name="ps_s", bufs=2, space="PSUM") as ps_s,
            tc.tile_pool(name="ps_o", bufs=1, space="PSUM") as ps_o,
            tc.tile_pool(name="ps_h", bufs=2, space="PSUM") as ps_h,
        ):
            # ---- load inputs; f and fT pieces interleaved by first use ----
            fb = singles.tile([C, M], bf16)
            fT = singles.tile([128, NCH, 128], bf16)  # [n_local, chunk, c]
            fT_flat = fT.rearrange("p a b -> p (a b)")

            def f_piece(q):
                nc.sync.dma_start(
                    out=fb[:, q * 512 : (q + 1) * 512],
                    in_=f_d[:, q * 512 : (q + 1) * 512],
                )

            def ft_piece(q):
                nc.sync.dma_start(
                    out=fT_flat[:, q * 512 : (q + 1) * 512],
                    in_=fT_d[:, q * 512 : (q + 1) * 512],
                )

            for q in [0, 1]:
                f_piece(q)
            ft_piece(0)
            for q in [2, 3]:
                f_piece(q)
            ft_piece(1)
            for q in [4, 5]:
                f_piece(q)
            ft_piece(2)
            for q in [6, 7]:
                f_piece(q)
            for q in range(3, 8):
                ft_piece(q)
            w1aT = singles.tile([C, C], f32r)
            nc.sync.dma_start(out=w1aT, in_=w1aT_d[:, :])
            w1bT = singles.tile([C, C], bf16)
            nc.sync.dma_start(out=w1bT, in_=w1bT_d[:, :])
            bhead = singles.tile([C, 1], f32)
            nc.sync.dma_start(out=bhead, in_=bhead_d[:, :])
            w2T = singles.tile([C, 1], f32r)
            nc.sync.dma_start(out=w2T, in_=w2T_d[:, :])
            sel = singles.tile([CPS, CPS * 128], f32r)
            nc.sync.dma_start(out=sel, in_=sel_d[:, :])

            # ---- identity for the Dinv transpose ----
            ident_f32 = singles.tile([128, 128], f32)
            make_identity(nc, ident_f32)

            # softmax denominator partials: pD[n_local, chunk, super]
            pD = singles.tile([128, NCH, NSUP], f32)
            O_sb = singles.tile([C, 3 * SUP], f32r)
            pred_sb = singles.tile([1, M], f32)

            drow_tiles = {}

            def head_d(g):
                """Softmax denominators for m-group g (DVE only)."""
                Dg = sbm.tile([128, CPS], f32, tag="Dg", name=f"Dg{g}")
                nc.vector.tensor_reduce(
                    out=Dg,
                    in_=pD[:, CPS * g : CPS * (g + 1), :],
                    axis=AX.X,
                    op=OP.add,
                )
                Dinvg = sbm.tile([128, CPS], f32, tag="Dinvg", name=f"Dinvg{g}")
                nc.vector.reciprocal(out=Dinvg, in_=Dg)
                drow_tiles[g] = Dinvg

            drpg_tiles = {}

            def head_t_pe(g):
                """Transpose Dinv for m-group g (PE half)."""
                pool, ptag = (ps_h, "ph")
                Dinvg = drow_tiles[g]
                drpg = pool.tile([CPS, 128], f32, tag=ptag, name=f"drpg{g}")
                nc.tensor.transpose(drpg, Dinvg, ident_f32)
                drpg_tiles[g] = drpg

            def head_t_copy(g):
                """Transpose Dinv for m-group g (DVE copy half)."""
                drpg = drpg_tiles.pop(g)
                DrowTg = sbm.tile([CPS, 128], f32r, tag="DrowTg", name=f"DrowTg{g}")
                nc.vector.tensor_copy(out=DrowTg, in_=drpg)
                drow_tiles[g] = DrowTg

            def head_t(g):
                head_t_pe(g)
                head_t_copy(g)

            fnorm_tiles = {}
            dinvb_tiles = {}

            def head_bcast(g, h, pool, ptag):
                """Dinv broadcast [128,512] via PE sel-matmul."""
                DrowTg = drow_tiles[g]
                dbp = pool.tile([128, 512], f32, tag=ptag, name=f"dbp{g}_{h}")
                for j in range(4):
                    jj = h * 4 + j
                    nc.tensor.matmul(
                        dbp[:, j * 128 : (j + 1) * 128],
                        lhsT=sel[:, jj * 128 : (jj + 1) * 128],
                        rhs=DrowTg,
                        start=True,
                        stop=True,
                    )
                return dbp

            def head_bcast3(g, h):
                """Group 3: broadcast in-loop (ps_h) + stage Dinv in SBUF,
                so the post-stream fnorm only waits on the final PV."""
                dbp = head_bcast(g, h, ps_h, "ph")
                dinvb = sbm.tile([128, 512], f32, tag="dinvb", name=f"dvb{h}")
                nc.vector.tensor_copy(out=dinvb, in_=dbp)
                dinvb_tiles[(g, h)] = dinvb

            def head_fnorm3(g, h):
                """Group 3: fnorm straight off the ot PSUM tile."""
                src0 = ot_tiles[g][:, h * 512 : (h + 1) * 512]
                fnorm = sbm.tile([128, 512], f32r, tag="fnorm", name=f"fn{g}_{h}")
                nc.vector.tensor_tensor(
                    out=fnorm, in0=src0, in1=dinvb_tiles.pop((g, h)), op=OP.mult
                )
                fnorm_tiles[(g, h)] = fnorm

            dbp_tiles = {}

            def head_fnormO(g, h, dbp):
                """fnorm from O_sb (groups 0..2)."""
                base = g * SUP + h * 512
                fnorm = sbm.tile([128, 512], f32r, tag="fnorm", name=f"fn{g}_{h}")
                nc.vector.tensor_tensor(
                    out=fnorm, in0=O_sb[:, base : base + 512], in1=dbp, op=OP.mult
                )
                fnorm_tiles[(g, h)] = fnorm

            def head_pre(g, h):
                """Groups 0..2: broadcast + fnorm from O_sb."""
                pool, ptag = (ps_s, "st") if g == 2 else (ps_h, "ph")
                dbp = head_bcast(g, h, pool, ptag)
                head_fnormO(g, h, dbp)

            zsb_tiles = {}

            def head_mid(g, h):
                """First conv (accumulated) + bias + leaky relu."""
                pool, ptag = (ps_s, "st") if g >= 2 else (ps_h, "ph")
                base = g * SUP + h * 512
                hsl = bass.ds(base, 512)
                yp = pool.tile([128, 512], f32, tag=ptag, name=f"yp{g}_{h}")
                nc.tensor.matmul(
                    yp, lhsT=w1aT, rhs=fnorm_tiles.pop((g, h)), start=True, stop=False
                )
                nc.tensor.matmul(yp, lhsT=w1bT, rhs=fb[:, hsl], start=False, stop=True)
                zsb = sbm.tile([128, 512], f32r, tag="zsb", name=f"zsb{g}_{h}")
                if g >= 2:
                    # tail groups: ACT is idle here, keep the fused Prelu
                    nc.scalar.activation(
                        out=zsb, in_=yp, func=AF.Prelu, bias=bhead, scale=1.0,
                        alpha=LEAKY,
                    )
                else:
                    # mid-loop: ACT is the bottleneck -- leaky on DVE
                    t1 = sbm.tile([128, 512], f32, tag="t1", name=f"t1_{g}_{h}")
                    nc.vector.tensor_scalar_add(out=t1, in0=yp, scalar1=bhead)
                    nc.vector.scalar_tensor_tensor(
                        out=zsb, in0=t1, scalar=LEAKY, in1=t1,
                        op0=OP.mult, op1=OP.max,
                    )
                zsb_tiles[(g, h)] = zsb

            def head_post(g, h):
                """Final 1-channel conv + pred copy + piecewise DMA out."""
                pool, ptag = (ps_s, "st") if g >= 2 else (ps_h, "ph")
                base = g * SUP + h * 512
                hsl = bass.ds(base, 512)
                pp = pool.tile([1, 512], f32, tag=ptag, name=f"pp{g}_{h}")
                nc.tensor.matmul(
                    pp, lhsT=w2T, rhs=zsb_tiles.pop((g, h)), start=True, stop=True
                )
                if g == 3:
                    # tail: ACT is idle; Prelu(alpha=1) is an ACT copy that
                    # stays in the already-loaded table set
                    nc.scalar.activation(
                        out=pred_sb[0:1, hsl], in_=pp, func=AF.Prelu, scale=1.0,
                        alpha=1.0,
                    )
                else:
                    nc.vector.tensor_copy(out=pred_sb[0:1, hsl], in_=pp)
                nc.sync.dma_start(out=pred_d[:, base : base + 512],
                                  in_=pred_sb[0:1, hsl])

            # ---- main attention loop ----
            # Global chunk stream with 1-chunk S-matmul lookahead so an
            # eviction-stalled PV never blocks the next S (and hence exp).
            # The last super's chunks run reordered so group 3's D partials
            # (and its Dinv broadcast) are finished in-loop, and group 2's D
            # completes exactly at stream end.
            s3_order = (
                list(range(24, 32)) + list(range(0, 16)) + list(range(16, 24))
            )
            seq = [(s, t) for s in range(NSUP - 1) for t in range(NCH)]
            seq += [(NSUP - 1, t) for t in s3_order]
            st_tiles = {}
            ot_tiles = {}

            def emit_s(idx):
                s, t = seq[idx]
                st = ps_s.tile([128, SUP], f32, tag="st", name=f"st{s}_{t}")
                for q in range(2):
                    nc.tensor.matmul(
                        st[:, q * 512 : (q + 1) * 512],
                        lhsT=fb[:, t * 128 : (t + 1) * 128],
                        rhs=fb[:, s * SUP + q * 512 : s * SUP + (q + 1) * 512],
                        start=True,
                        stop=True,
                    )
                st_tiles[(s, t)] = st

            def emit_pv(s, t, p, pb):
                """PV for chunk (s,t); lags the exp stream by one chunk so
                the next chunk's S-matmuls reach the head of the PE queue
                the moment their st slot frees (keeps ACT fed)."""
                if p == 0:
                    ot_tiles[s] = ps_o.tile([C, SUP], f32, tag="ot", name=f"ot{s}")
                ot = ot_tiles[s]
                for q in range(2):
                    nc.tensor.matmul(
                        ot[:, q * 512 : (q + 1) * 512],
                        lhsT=fT[:, t, :],
                        rhs=pb[:, q * 512 : (q + 1) * 512],
                        start=(p == 0),
                        stop=(p == NCH - 1),
                    )
                if p == NCH - 1 and s < NSUP - 1:
                    for q in range(2):
                        nc.vector.tensor_copy(
                            out=O_sb[:, s * SUP + q * 512 : s * SUP + (q + 1) * 512],
                            in_=ot[:, q * 512 : (q + 1) * 512],
                        )

            emit_s(0)
            pv_pending = None
            for i, (s, t) in enumerate(seq):
                p = i % NCH  # position within the super
                st = st_tiles.pop((s, t))
                pb = pbufp.tile([128, SUP], bf16, tag="pb", name=f"pb{s}_{t}")
                if _act_accum(s, t):
                    nc.scalar.activation(
                        out=pb, in_=st, func=AF.Exp, scale=SCALE,
                        accum_out=pD[:, t, s : s + 1],
                    )
                else:
                    nc.scalar.activation(out=pb, in_=st, func=AF.Exp, scale=SCALE)
                if i + 1 < len(seq):
                    emit_s(i + 1)
                if pv_pending is not None:
                    emit_pv(*pv_pending)
                pv_pending = (s, t, p, pb)
                if not _act_accum(s, t):
                    nc.vector.tensor_reduce(
                        out=pD[:, t, s : s + 1], in_=pb, axis=AX.X, op=OP.add
                    )
                if s == NSUP - 1:
                    if p == 8:
                        head_d(3)
                    elif p == 9:
                        head_t(3)
                    elif p == 10:
                        head_bcast3(3, 0)
                    elif p == 11:
                        head_bcast3(3, 1)
                    elif p == 16:
                        head_d(0)
                    elif p == 17:
                        head_t(0)
                    elif p == 18:
                        head_pre(0, 0)
                    elif p == 19:
                        head_pre(0, 1)
                    elif p == 20:
                        head_mid(0, 0)
                    elif p == 21:
                        head_mid(0, 1)
                    elif p == 22:
                        head_post(0, 0)
                    elif p == 23:
                        head_post(0, 1)
                    elif p == 24:
                        head_d(1)
                    elif p == 25:
                        head_t(1)
                    elif p == 26:
                        head_pre(1, 0)
                    elif p == 27:
                        head_pre(1, 1)
                    elif p == 28:
                        head_mid(1, 0)
                    elif p == 29:
                        head_mid(1, 1)
                        head_post(1, 0)
                    elif p == 30:
                        head_post(1, 1)

            # ---- tail: group 3 (fnorm gated only by the final PV) and
            # group 2 (D-chain gated by the final accums). Emission order
            # tuned so the critical DVE chain (fnorm3 -> yp3 -> Prelu3) and
            # PE chain are never queued behind group 2's slower pieces ----
            emit_pv(*pv_pending)
            head_d(2)
            head_t_pe(2)
            head_fnorm3(3, 0)
            head_t_copy(2)
            head_fnorm3(3, 1)
            head_mid(3, 0)
            dbp2_0 = head_bcast(2, 0, ps_h, "ph")
            head_mid(3, 1)
            dbp2_1 = head_bcast(2, 1, ps_h, "ph")
            head_fnormO(2, 0, dbp2_0)
            head_fnormO(2, 1, dbp2_1)
            head_mid(2, 0)
            head_mid(2, 1)
            head_post(3, 0)
            head_post(3, 1)
            head_post(2, 0)
            head_post(2, 1)

    nc.finalize()
    return nc


_NC = None


def _get_nc():
    global _NC
    if _NC is None:
        _NC = _build()
    return _NC


def _prepare_in_maps(inputs):
    curr = np.asarray(inputs["curr_features"], np.float32)
    w1 = np.asarray(inputs["w1"], np.float32)
    b1 = np.asarray(inputs["b1"], np.float32)
    gamma = np.asarray(inputs["gamma"], np.float32)
    beta = np.asarray(inputs["beta"], np.float32)
    rm = np.asarray(inputs["running_mean"], np.float32)
    rv = np.asarray(inputs["running_var"], np.float32)
    w2 = np.asarray(inputs["w2"], np.float32)

    # fold BN (inference) into the first conv
    a = gamma / np.sqrt(rv + BN_EPS)                      # [C]
    W1f = w1 * a[:, None]                                 # [C, 2C]
    bhead = (b1 * a + beta - rm * a).astype(np.float32).reshape(C, 1)
    w1aT = np.ascontiguousarray(W1f[:, :C].T, np.float32)  # feats1 part
    w1bT = np.ascontiguousarray(W1f[:, C:].T).astype(ml_dtypes.bfloat16)
    w2T = np.ascontiguousarray(w2.T, np.float32)           # [C, 1]

    selm = np.zeros((CPS, CPS * 128), np.float32)
    for k in range(CPS):
        selm[k, k * 128 : (k + 1) * 128] = 1.0

    in_maps = []
    for b in range(B):
        in_maps.append(
            {
                "f": np.ascontiguousarray(curr[b].reshape(C, M)).astype(
                    ml_dtypes.bfloat16
                ),
                "fT": np.ascontiguousarray(
                    curr[b].reshape(C, NCH, 128).transpose(2, 1, 0).reshape(
                        128, NCH * 128
                    )
                ).astype(ml_dtypes.bfloat16),
                "w1aT": w1aT,
                "w1bT": w1bT,
                "bhead": bhead,
                "w2T": w2T,
                "sel": selm,
            }
        )
    return in_maps


def kernel(**inputs):
    b2 = np.asarray(inputs["b2"], np.float32)
    nc = _get_nc()
    in_maps = _prepare_in_maps(inputs)
    res = run_bass_kernel_spmd(nc, in_maps, core_ids=list(range(B)))
    preds = np.stack([r["pred"].reshape(1, H, W) for r in res.results], axis=0)
    return (preds + b2[0]).astype(np.float32)


if __name__ == "__main__":
    _build()
    print("build OK")
